# revision 23
# baseline (speedup 1.0000x reference)
"""Trainium2 Bass kernel for nn_Decoder (attention-LSTM decoder recurrence).

Math (per batch b, T=128 steps, M=P=64):
    UH = H @ U_d.T                                  (hoisted)
    repeat t = 0..T-2:
        q = [h; c]                                  (128,)
        e = tanh(UH + W_d @ q)                      (T, M)
        s = exp(v_d . e)                            (T,)   softmax numerator
        den = sum_t s_t;  num' = sum_t s_t * hw'_t
          where hw'_t[tau] = (H w~[1:] + w~b)[tau] + w~0*dec_t   (precomputed)
        y~  = num' / den                            (decoder input folded in)
        LSTM(y~, h, c) -> h, c                      (gate col order f,g,i,o)
    final: attend once more; out = [h, ctx]

Sharding: data-parallel over batch. B=32 over 8 cores -> 4 batches/core.

Key structure: E = UH + W_d q is built in PSUM by the tensor engine
(identity matmul copies UH f16 from SBUF, then 8 broadcast-rhs matmuls
accumulate the per-batch bias into each [64, T] quadrant), so ONE
bias-free tanh [128, 2T] replaces two biased tanh ops + psum->sbuf copy.
Gates are split whh-part (off-chain, needs only h) / y-part (on-chain,
tiny [2,64] stationaries at partition base 64).

Layout rules (walrus): non-matmul ops need all operands at the same start
partition; partition bases restricted to {0, 32, 64}; custom DVE ops and
tensor_tensor(divide) do NOT work in this runtime.  State stores 2h / 2c
(sigmoid(z) = 0.5 tanh(0.5 z) + 0.5 folded into host-packed weights).
"""

import contextlib

import numpy as np

B, T, M, P = 32, 128, 64, 64
NCORES = 8
BL = B // NCORES          # batches per core = 4
NG = 2                    # attention groups per core
GB = BL // NG             # batches per attention group = 2
TS = T - 1                # recurrence steps = 127

_STATE = {}


def _build_nc():
    import concourse.bacc as bacc
    import concourse.tile as tile
    from concourse import mybir

    f32 = mybir.dt.float32
    f32r = mybir.dt.float32r
    f16 = mybir.dt.float16
    AF = mybir.ActivationFunctionType
    OP = mybir.AluOpType

    nc = bacc.Bacc()

    # ---- per-core sharded data ----
    h_l = nc.declare_dram_parameter("h_l", [BL, T, M], f32, isOutput=False)
    ht_l = nc.declare_dram_parameter("ht_l", [BL, M, T], f32, isOutput=False)
    st0 = nc.declare_dram_parameter("st0", [2 * P, BL], f16, isOutput=False)
    hwf = nc.declare_dram_parameter("hwf", [T, BL * TS], f16, isOutput=False)
    # ---- replicated packed weights ----
    wdh = nc.declare_dram_parameter("wdh", [P, M], f16, isOutput=False)
    wdc = nc.declare_dram_parameter("wdc", [P, M], f16, isOutput=False)
    udT = nc.declare_dram_parameter("udT", [M, M], f32, isOutput=False)
    v2 = nc.declare_dram_parameter("v2", [2 * M, GB], f16, isOutput=False)
    whha = nc.declare_dram_parameter("whha", [P + 2, 4 * P], f16, isOutput=False)
    eye = nc.declare_dram_parameter("eye", [2 * M, 2 * M], f16, isOutput=False)
    # ---- outputs ----
    oh = nc.declare_dram_parameter("oh", [P, BL], f32, isOutput=True)
    octx = nc.declare_dram_parameter("octx", [M + 1, BL], f32, isOutput=True)

    with tile.TileContext(nc) as tc:
        with contextlib.ExitStack() as ctx:
            consts = ctx.enter_context(tc.tile_pool(name="consts", bufs=1))
            state = ctx.enter_context(tc.tile_pool(name="state", bufs=1))
            loop_sb = ctx.enter_context(tc.tile_pool(name="loop_sb", bufs=3))
            loop_ps = ctx.enter_context(
                tc.tile_pool(name="loop_ps", bufs=1, space="PSUM")
            )

            # ---------------- preamble: constants ----------------
            wdh_sb = consts.tile([P, M], f16)
            nc.sync.dma_start(out=wdh_sb, in_=wdh[:])
            wdc_sb = consts.tile([P, M], f16)
            nc.sync.dma_start(out=wdc_sb, in_=wdc[:])
            udT_sb = consts.tile([M, M], f32)
            nc.sync.dma_start(out=udT_sb, in_=udT[:])
            v2_sb = consts.tile([2 * M, GB], f16)
            nc.sync.dma_start(out=v2_sb, in_=v2[:])
            whha_sb = consts.tile([P + 2, 4 * P], f16)
            nc.sync.dma_start(out=whha_sb, in_=whha[:])
            eye_sb = consts.tile([2 * M, 2 * M], f16)
            nc.sync.dma_start(out=eye_sb, in_=eye[:])
            hwfull = consts.tile([T, BL * TS], f16)
            nc.sync.dma_start(out=hwfull, in_=hwf[:])
            ones_f = consts.tile([T, 1], f32)
            nc.vector.memset(ones_f, 1.0)
            ones_col = consts.tile([T, 1], f16)
            nc.vector.tensor_copy(out=ones_col, in_=ones_f)

            # state: sy = [2h (0:64); y~ (64); 1 (65)]
            sy = state.tile([66, BL], f16, tag="SY")
            ones_g = state.tile([66, BL], f32, tag="ONESG")
            nc.vector.memset(ones_g[64:66, :], 1.0)
            nc.vector.tensor_copy(out=sy[64:66, :], in_=ones_g[64:66, :])
            nc.sync.dma_start(out=sy[0:P, :], in_=st0[0:P, :])
            sh = sy[0:P, :]
            # TGC: [2c (0:4) | tf (4:8) | tg (8:12) | ti (12:16) | to (16:20)]
            TGC = state.tile([P, 20], f16, tag="TGC")
            nc.sync.dma_start(out=TGC[:, 0:4], in_=st0[P:2 * P, :])

            # ht tiles
            ht_tiles = []
            for b in range(BL):
                htb = consts.tile([M, T], f32, tag=f"HT{b}")
                nc.sync.dma_start(out=htb, in_=ht_l[b])
                ht_tiles.append(htb)

            # UH = H @ U_d.T -> f16 SBUF [ (j,m), (g, tau) ]
            UH_sb = consts.tile([2 * M, NG * T], f16)
            with tc.tile_pool(name="uhps", bufs=1, space="PSUM") as uhp:
                uh = uhp.tile([2 * M, NG * T], f32)
                for g in range(NG):
                    for j in range(GB):
                        nc.tensor.matmul(
                            uh[j * M:(j + 1) * M, g * T:(g + 1) * T], udT_sb,
                            ht_tiles[g * GB + j], start=True, stop=True,
                        )
                nc.vector.tensor_copy(out=UH_sb, in_=uh)

            hw_v = hwfull.rearrange("p (b ts) -> p b ts", b=BL)

            # H tiles for the final context matmul
            HAUG = []
            for b in range(BL):
                haug = state.tile([T, M], f32r, tag=f"HAUG{b}")
                nc.sync.dma_start(out=haug, in_=h_l[b].bitcast(f32r))
                HAUG.append(haug)

            def attention(t, last=False):
                # E = UH + W_d q, built on the tensor engine in PSUM
                # (parity double-buffer: PE rebuilds E while Act reads the
                # previous step's buffer)
                E = loop_ps.tile([2 * M, NG * T], f32, tag=f"E{t % 2}")
                nc.tensor.matmul(E, eye_sb, UH_sb, start=True, stop=False)
                for g in range(NG):
                    for j in range(GB):
                        b = g * GB + j
                        nc.tensor.matmul(
                            E[j * M:(j + 1) * M, g * T:(g + 1) * T], wdc_sb,
                            TGC[:, b:b + 1].to_broadcast([P, T]),
                            start=False, stop=False)
                for g in range(NG):
                    for j in range(GB):
                        b = g * GB + j
                        nc.tensor.matmul(
                            E[j * M:(j + 1) * M, g * T:(g + 1) * T], wdh_sb,
                            sh[:, b:b + 1].to_broadcast([P, T]),
                            start=False, stop=True)
                gps = None
                e_sb = loop_sb.tile([2 * M, NG * T], f16, tag="e")
                nc.scalar.activation(out=e_sb, in_=E, func=AF.Tanh)
                lg = loop_ps.tile([T, BL], f32, tag="LG")
                for g in range(NG):
                    nc.tensor.matmul(lg[:, g * GB:(g + 1) * GB],
                                     e_sb[:, g * T:(g + 1) * T], v2_sb,
                                     start=True, stop=True)
                sss = loop_sb.tile([T, 2 * BL], f16, tag="sss")
                nc.scalar.activation(out=sss[:, 0:BL], in_=lg, func=AF.Exp)
                ndps = loop_ps.tile([66, 2 * BL], f32, tag="ND")
                if not last:
                    nc.vector.tensor_tensor(
                        out=sss[:, BL:2 * BL], in0=sss[:, 0:BL],
                        in1=hw_v[:, :, t], op=OP.mult)
                    nc.tensor.matmul(ndps[64:65, :], ones_col, sss,
                                     start=True, stop=True)
                else:
                    nc.tensor.matmul(ndps[64:65, 0:BL], ones_col,
                                     sss[:, 0:BL], start=True, stop=True)
                return sss, ndps, gps

            # ---------------- main recurrence ----------------
            for t in range(TS):
                sss, ndps, gps = attention(t)
                # y~ = num'/den -> sy[64]
                rden = loop_sb.tile([66, BL], f32, tag="rden")
                nc.vector.reciprocal(out=rden[64:65, :],
                                     in_=ndps[64:65, 0:BL])
                nc.vector.tensor_tensor(
                    out=sy[64:65, :], in0=ndps[64:65, BL:2 * BL],
                    in1=rden[64:65, :], op=OP.mult)
                # gates
                gps = loop_ps.tile([P, 4 * BL], f32, tag="G")
                for k in range(4):
                    nc.tensor.matmul(
                        gps[:, k * BL:(k + 1) * BL],
                        whha_sb[:, k * P:(k + 1) * P], sy,
                        start=True, stop=True)
                nc.scalar.activation(out=TGC[:, 4:20], in_=gps, func=AF.Tanh)
                # sUV = [(tf+1)*2c | (ti+1)*tg]
                fi = TGC[:, 4:20].rearrange(
                    "p (a c k) -> p a c k", a=2, c=2)[:, :, 0, :]
                cg = TGC[:, 0:16].rearrange(
                    "p (a c k) -> p a c k", a=2, c=2)[:, :, 0, :]
                sUV = loop_sb.tile([P, 2 * BL], f16, tag="sUV")
                nc.vector.scalar_tensor_tensor(
                    out=sUV.rearrange("p (a k) -> p a k", a=2),
                    in0=fi, scalar=1.0, in1=cg, op0=OP.add, op1=OP.mult)
                # 2c' = 0.5*sU + sV
                nc.vector.scalar_tensor_tensor(
                    out=TGC[:, 0:4], in0=sUV[:, 0:BL], scalar=0.5,
                    in1=sUV[:, BL:2 * BL], op0=OP.mult, op1=OP.add)
                th = loop_sb.tile([P, BL], f16, tag="th")
                nc.scalar.activation(out=th, in_=TGC[:, 0:4], func=AF.Tanh,
                                     scale=0.5)
                # 2h' = (to+1)*th
                nc.vector.scalar_tensor_tensor(
                    out=sh, in0=TGC[:, 16:20], scalar=1.0, in1=th,
                    op0=OP.add, op1=OP.mult)

            # ---------------- final attend + outputs ----------------
            sss, ndps, _ = attention(TS, last=True)
            s_fr = loop_sb.tile([T, BL], f32r, tag="sfr")
            nc.vector.tensor_copy(out=s_fr, in_=sss[:, 0:BL])
            ctx_ps = loop_ps.tile([M, 2 * BL], f32, tag="CTXF")
            for b in range(BL):
                nc.tensor.matmul(
                    ctx_ps[:, 2 * b:2 * b + 2].bitcast(f32),
                    HAUG[b], s_fr[:, b:b + 1].to_broadcast([T, 2]),
                    start=True, stop=True)
            ctx_out = loop_sb.tile([M + 1, BL], f32, tag="ctxout")
            nc.vector.tensor_copy(
                out=ctx_out[0:M, :],
                in_=ctx_ps.rearrange("p (b two) -> p b two", two=2)[:, :, 0])
            nc.vector.tensor_copy(
                out=ctx_out[64:65, :], in_=ndps[64:65, 0:BL])
            nc.sync.dma_start(out=octx[:], in_=ctx_out)
            nc.gpsimd.dma_start(out=oh[:], in_=sh)

    nc.finalize()
    return nc


def _pack_weights(W_d, U_d, v_d, w_tilde_W, w_tilde_b, W_ih, W_hh, b_ih, b_hh):
    f = np.float32
    # q = [h;c] stored as 2h;2c -> fold 0.5 into W_d^T halves
    wdT = 0.5 * W_d.T  # [128, 64]
    wdh = np.ascontiguousarray(wdT[0:P, :], dtype=np.float16)
    wdc = np.ascontiguousarray(wdT[P:2 * P, :], dtype=np.float16)
    udT = np.ascontiguousarray(U_d.T, dtype=f)                  # [64, 64]
    v2 = np.zeros((2 * M, GB), dtype=np.float16)
    v2[0:M, 0] = v_d[0]
    v2[M:2 * M, 1] = v_d[0]
    bsum = (b_ih + b_hh).astype(f)
    wih = W_ih[:, 0].astype(f)
    # torch gate order i,f,g,o; our column order f,g,i,o.
    # sigmoid gates (i,f,o): pre-scale 0.5 (sigmoid(z) = 0.5 tanh(0.5 z)+0.5)
    # h input is 2h -> extra 0.5 on W_hh blocks.
    src = [1, 2, 0, 3]                    # f, g, i, o row-blocks in torch order
    sig = [0.5, 1.0, 0.5, 0.5]
    whha = np.zeros((P + 2, 4 * P), dtype=np.float16)
    for k in range(4):
        blk = slice(src[k] * P, (src[k] + 1) * P)
        whha[0:P, k * P:(k + 1) * P] = sig[k] * 0.5 * W_hh[blk].T
        whha[P, k * P:(k + 1) * P] = sig[k] * wih[blk]
        whha[P + 1, k * P:(k + 1) * P] = sig[k] * bsum[blk]
    eye = np.eye(2 * M, dtype=np.float16)
    return dict(wdh=wdh, wdc=wdc, udT=udT, v2=v2, whha=whha, eye=eye)


def kernel(H, dec_data, d_1, s_1, W_d, U_d, v_d, w_tilde_W, w_tilde_b,
           W_ih, W_hh, b_ih, b_hh, T=None):
    from concourse.bass_utils import run_bass_kernel_spmd

    H = np.asarray(H, dtype=np.float32)
    dec_data = np.asarray(dec_data, dtype=np.float32)
    d_1 = np.asarray(d_1, dtype=np.float32)
    s_1 = np.asarray(s_1, dtype=np.float32)
    W_d = np.asarray(W_d, np.float32)
    w_tilde_W = np.asarray(w_tilde_W, np.float32)
    w_tilde_b = np.asarray(w_tilde_b, np.float32)

    if "nc" not in _STATE:
        _STATE["nc"] = _build_nc()
    nc = _STATE["nc"]

    wpack = _pack_weights(
        W_d, np.asarray(U_d, np.float32),
        np.asarray(v_d, np.float32), w_tilde_W, w_tilde_b,
        np.asarray(W_ih, np.float32), np.asarray(W_hh, np.float32),
        np.asarray(b_ih, np.float32), np.asarray(b_hh, np.float32),
    )

    w1 = w_tilde_W[0, 1:M + 1]            # [64]
    w0 = w_tilde_W[0, 0]
    wb = w_tilde_b[0]

    in_maps = []
    for core in range(NCORES):
        sl = slice(core * BL, (core + 1) * BL)
        h_loc = np.ascontiguousarray(H[sl])
        ht_loc = np.ascontiguousarray(h_loc.transpose(0, 2, 1))
        st = np.concatenate(
            [2.0 * d_1[0, sl].T, 2.0 * s_1[0, sl].T], axis=0
        ).astype(np.float16)                       # [128, 4]
        # hwf[tau, b*TS+ts] = (H_b @ w~1 + w~b) [tau] + w~0*dec[b, ts]
        hwb = h_loc @ w1 + wb                      # [BL, T]
        hwfa = (hwb[:, None, :] + (w0 * dec_data[sl, :TS, 0])[:, :, None])
        hwfa = np.ascontiguousarray(
            hwfa.transpose(2, 0, 1).reshape(T, BL * TS), dtype=np.float16)
        m = dict(wpack)
        m.update(h_l=h_loc, ht_l=ht_loc, st0=np.ascontiguousarray(st),
                 hwf=hwfa)
        in_maps.append(m)

    res = run_bass_kernel_spmd(nc, in_maps, list(range(NCORES)))
    _STATE["last_results"] = res

    out = np.zeros((B, 1, P + M), dtype=np.float32)
    for core in range(NCORES):
        r = res.results[core]
        hv = r["oh"].T * 0.5                      # [4, 64]  (state was 2h)
        octx = r["octx"]
        ctx = (octx[0:M] / octx[M:M + 1]).T       # [4, 64]
        out[core * BL:(core + 1) * BL, 0, 0:P] = hv
        out[core * BL:(core + 1) * BL, 0, P:P + M] = ctx
    return out


# revision 24
# speedup vs baseline: 1.1160x; 1.1160x over previous
"""Trainium2 Bass kernel for nn_Decoder (attention-LSTM decoder recurrence).

Math (per batch b, T=128 steps, M=P=64):
    UH = H @ U_d.T                                  (hoisted)
    repeat t = 0..T-2:
        q = [h; c]                                  (128,)
        e = tanh(UH + W_d @ q)                      (T, M)
        s = exp(v_d . e)                            (T,)   softmax numerator
        den = sum_t s_t;  num' = sum_t s_t * hw'_t
          where hw'_t[tau] = (H w~[1:] + w~b)[tau] + w~0*dec_t   (precomputed)
        y~  = num' / den                            (decoder input folded in)
        LSTM(y~, h, c) -> h, c                      (gate col order f,g,i,o)
    final: attend once more; out = [h, ctx]

Sharding: data-parallel over batch. B=32 over 8 cores -> 4 batches/core.

Layout rules (walrus): non-matmul ops need all operands at the same start
partition; partition bases restricted to {0, 32, 64}; fp32r matmuls need
even innermost free count + psum partition 0.  State stores 2h / 2c
(sigmoid(z) = 0.5 tanh(0.5 z) + 0.5 folded into host-packed weights).

Step chain (one 4-batch chain):
  PE qW(4mm->qwps[128,2]) -> DVE copy -> Act tanh-e x2 -> PE logits x2
  -> Act exp -> GpSimd mult (s*hw') -> PE nd(1mm->[1,8]@p64) -> DVE divide
  -> PE gates x4 -> Act tanh16 -> DVE sUV,c' -> Act th -> DVE h'
"""

import contextlib

import numpy as np

B, T, M, P = 32, 128, 64, 64
NCORES = 8
BL = B // NCORES          # batches per core = 4
NG = 2                    # attention groups per core
GB = BL // NG             # batches per attention group = 2
TS = T - 1                # recurrence steps = 127

_STATE = {}


def _build_nc():
    import concourse.bacc as bacc
    import concourse.tile as tile
    from concourse import mybir

    f32 = mybir.dt.float32
    f32r = mybir.dt.float32r
    f16 = mybir.dt.float16
    AF = mybir.ActivationFunctionType
    OP = mybir.AluOpType

    nc = bacc.Bacc()

    # ---- per-core sharded data ----
    h_l = nc.declare_dram_parameter("h_l", [BL, T, M], f32, isOutput=False)
    ht_l = nc.declare_dram_parameter("ht_l", [BL, M, T], f32, isOutput=False)
    st0 = nc.declare_dram_parameter("st0", [2 * P, BL], f16, isOutput=False)
    hwf = nc.declare_dram_parameter("hwf", [T, BL * TS], f16, isOutput=False)
    # ---- replicated packed weights ----
    wdh = nc.declare_dram_parameter("wdh", [P, M], f16, isOutput=False)
    wdc = nc.declare_dram_parameter("wdc", [P, M], f16, isOutput=False)
    udT = nc.declare_dram_parameter("udT", [M, M], f32, isOutput=False)
    v2 = nc.declare_dram_parameter("v2", [2 * M, GB], f16, isOutput=False)
    whha = nc.declare_dram_parameter("whha", [P + 2, 4 * P], f16, isOutput=False)
    # ---- outputs ----
    oh = nc.declare_dram_parameter("oh", [P, BL], f32, isOutput=True)
    octx = nc.declare_dram_parameter("octx", [M + 1, BL], f32, isOutput=True)

    with tile.TileContext(nc) as tc:
        with contextlib.ExitStack() as ctx:
            consts = ctx.enter_context(tc.tile_pool(name="consts", bufs=1))
            state = ctx.enter_context(tc.tile_pool(name="state", bufs=1))
            loop_sb = ctx.enter_context(tc.tile_pool(name="loop_sb", bufs=3))
            loop_ps = ctx.enter_context(
                tc.tile_pool(name="loop_ps", bufs=1, space="PSUM")
            )
            uh_pool = ctx.enter_context(
                tc.tile_pool(name="uh_ps", bufs=1, space="PSUM")
            )

            # ---------------- preamble: constants ----------------
            wdh_sb = consts.tile([P, M], f16)
            nc.sync.dma_start(out=wdh_sb, in_=wdh[:])
            wdc_sb = consts.tile([P, M], f16)
            nc.sync.dma_start(out=wdc_sb, in_=wdc[:])
            udT_sb = consts.tile([M, M], f32)
            nc.sync.dma_start(out=udT_sb, in_=udT[:])
            v2_sb = consts.tile([2 * M, GB], f16)
            nc.sync.dma_start(out=v2_sb, in_=v2[:])
            whha_sb = consts.tile([P + 2, 4 * P], f16)
            nc.sync.dma_start(out=whha_sb, in_=whha[:])
            hwfull = consts.tile([T, BL * TS], f16)
            nc.sync.dma_start(out=hwfull, in_=hwf[:])
            ones_f = consts.tile([T, 1], f32)
            nc.vector.memset(ones_f, 1.0)
            ones_col = consts.tile([T, 1], f16)
            nc.vector.tensor_copy(out=ones_col, in_=ones_f)

            # state: sy = [2h (0:64); y~ (64); 1 (65)]
            sy = state.tile([66, BL], f16, tag="SY")
            ones_g = state.tile([66, BL], f32, tag="ONESG")
            nc.vector.memset(ones_g[64:66, :], 1.0)
            nc.vector.tensor_copy(out=sy[64:66, :], in_=ones_g[64:66, :])
            nc.sync.dma_start(out=sy[0:P, :], in_=st0[0:P, :])
            # TGC: [2c (0:4) | tf (4:8) | tg (8:12) | ti (12:16) | to (16:20)]
            TGC = state.tile([P, 20], f16, tag="TGC")
            nc.sync.dma_start(out=TGC[:, 0:4], in_=st0[P:2 * P, :])

            # ht tiles
            ht_tiles = []
            for b in range(BL):
                htb = consts.tile([M, T], f32, tag=f"HT{b}")
                nc.sync.dma_start(out=htb, in_=ht_l[b])
                ht_tiles.append(htb)

            # UH = H @ U_d.T -> [(j,m), t] per group; PSUM-resident
            UH_ps = []
            for g in range(NG):
                uh = uh_pool.tile([2 * M, T], f32, tag=f"UH{g}")
                for j in range(GB):
                    nc.tensor.matmul(
                        uh[j * M:(j + 1) * M, :], udT_sb,
                        ht_tiles[g * GB + j], start=True, stop=True,
                    )
                UH_ps.append(uh)

            hw_v = hwfull.rearrange("p (b ts) -> p b ts", b=BL)

            # H tiles for the final context matmul
            HAUG = []
            for b in range(BL):
                haug = state.tile([T, M], f32r, tag=f"HAUG{b}")
                nc.sync.dma_start(out=haug, in_=h_l[b].bitcast(f32r))
                HAUG.append(haug)

            sy_h2 = sy[0:P, :].rearrange("p (g j) -> p j g", j=GB)
            cs_2 = TGC[:, 0:4].rearrange("p (g j) -> p j g", j=GB)

            def attention(t, last=False):
                # qW: 4 matmuls -> qwps [128, 2] (cols = groups)
                qwps = loop_ps.tile([2 * M, NG], f32, tag="QW")
                for j in range(GB):
                    nc.tensor.matmul(qwps[j * M:(j + 1) * M, :], wdc_sb,
                                     cs_2[:, j, :], start=True, stop=False)
                gps = None
                for j in range(GB):
                    nc.tensor.matmul(qwps[j * M:(j + 1) * M, :], wdh_sb,
                                     sy_h2[:, j, :], start=False, stop=True)
                qw_sb = loop_sb.tile([2 * M, NG], f32, tag="qw")
                nc.vector.tensor_copy(out=qw_sb, in_=qwps)
                # e = tanh(UH + qw)
                lg = loop_ps.tile([T, BL], f32, tag="LG")
                for g in range(NG):
                    e_sb = loop_sb.tile([2 * M, T], f16, tag=f"e{g}")
                    nc.scalar.activation(out=e_sb, in_=UH_ps[g], func=AF.Tanh,
                                         bias=qw_sb[:, g:g + 1], scale=1.0)
                    nc.tensor.matmul(lg[:, g * GB:(g + 1) * GB], e_sb, v2_sb,
                                     start=True, stop=True)
                sss = loop_sb.tile([T, 2 * BL], f16, tag="sss")
                nc.scalar.activation(out=sss[:, 0:BL], in_=lg, func=AF.Exp)
                ndps = loop_ps.tile([66, 2 * BL], f32, tag="ND")
                if not last:
                    nc.vector.tensor_tensor(
                        out=sss[:, BL:2 * BL], in0=sss[:, 0:BL],
                        in1=hw_v[:, :, t], op=OP.mult)
                    nc.tensor.matmul(ndps[64:65, :], ones_col, sss,
                                     start=True, stop=True)
                else:
                    nc.tensor.matmul(ndps[64:65, 0:BL], ones_col,
                                     sss[:, 0:BL], start=True, stop=True)
                return sss, ndps, gps

            # ---------------- main recurrence ----------------
            for t in range(TS):
                sss, ndps, gps = attention(t)
                # y~ = num'/den -> sy[64]
                rden = loop_sb.tile([66, BL], f32, tag="rden")
                nc.vector.reciprocal(out=rden[64:65, :],
                                     in_=ndps[64:65, 0:BL])
                nc.vector.tensor_tensor(
                    out=sy[64:65, :], in0=ndps[64:65, BL:2 * BL],
                    in1=rden[64:65, :], op=OP.mult)
                # gates
                gps = loop_ps.tile([P, 4 * BL], f32, tag="G")
                for k in range(4):
                    nc.tensor.matmul(
                        gps[:, k * BL:(k + 1) * BL],
                        whha_sb[:, k * P:(k + 1) * P], sy,
                        start=True, stop=True)
                nc.scalar.activation(out=TGC[:, 4:20], in_=gps, func=AF.Tanh)
                # sUV = [(tf+1)*2c | (ti+1)*tg]
                fi = TGC[:, 4:20].rearrange(
                    "p (a c k) -> p a c k", a=2, c=2)[:, :, 0, :]
                cg = TGC[:, 0:16].rearrange(
                    "p (a c k) -> p a c k", a=2, c=2)[:, :, 0, :]
                sUV = loop_sb.tile([P, 2 * BL], f16, tag="sUV")
                nc.vector.scalar_tensor_tensor(
                    out=sUV.rearrange("p (a k) -> p a k", a=2),
                    in0=fi, scalar=1.0, in1=cg, op0=OP.add, op1=OP.mult)
                # 2c' = 0.5*sU + sV
                nc.vector.scalar_tensor_tensor(
                    out=TGC[:, 0:4], in0=sUV[:, 0:BL], scalar=0.5,
                    in1=sUV[:, BL:2 * BL], op0=OP.mult, op1=OP.add)
                th = loop_sb.tile([P, BL], f16, tag="th")
                nc.scalar.activation(out=th, in_=TGC[:, 0:4], func=AF.Tanh,
                                     scale=0.5)
                # 2h' = (to+1)*th
                nc.vector.scalar_tensor_tensor(
                    out=sy[0:P, :], in0=TGC[:, 16:20], scalar=1.0, in1=th,
                    op0=OP.add, op1=OP.mult)

            # ---------------- final attend + outputs ----------------
            sss, ndps, _ = attention(TS, last=True)
            s_fr = loop_sb.tile([T, BL], f32r, tag="sfr")
            nc.vector.tensor_copy(out=s_fr, in_=sss[:, 0:BL])
            ctx_ps = loop_ps.tile([M, 2 * BL], f32, tag="CTXF")
            for b in range(BL):
                nc.tensor.matmul(
                    ctx_ps[:, 2 * b:2 * b + 2].bitcast(f32),
                    HAUG[b], s_fr[:, b:b + 1].to_broadcast([T, 2]),
                    start=True, stop=True)
            ctx_out = loop_sb.tile([M + 1, BL], f32, tag="ctxout")
            nc.vector.tensor_copy(
                out=ctx_out[0:M, :],
                in_=ctx_ps.rearrange("p (b two) -> p b two", two=2)[:, :, 0])
            nc.vector.tensor_copy(
                out=ctx_out[64:65, :], in_=ndps[64:65, 0:BL])
            nc.sync.dma_start(out=octx[:], in_=ctx_out)
            nc.gpsimd.dma_start(out=oh[:], in_=sy[0:P, :])

    nc.finalize()
    return nc


def _pack_weights(W_d, U_d, v_d, w_tilde_W, w_tilde_b, W_ih, W_hh, b_ih, b_hh):
    f = np.float32
    # q = [h;c] stored as 2h;2c -> fold 0.5 into W_d^T halves
    wdT = 0.5 * W_d.T  # [128, 64]
    wdh = np.ascontiguousarray(wdT[0:P, :], dtype=np.float16)
    wdc = np.ascontiguousarray(wdT[P:2 * P, :], dtype=np.float16)
    udT = np.ascontiguousarray(U_d.T, dtype=f)                  # [64, 64]
    v2 = np.zeros((2 * M, GB), dtype=np.float16)
    v2[0:M, 0] = v_d[0]
    v2[M:2 * M, 1] = v_d[0]
    bsum = (b_ih + b_hh).astype(f)
    wih = W_ih[:, 0].astype(f)
    # torch gate order i,f,g,o; our column order f,g,i,o.
    # sigmoid gates (i,f,o): pre-scale 0.5 (sigmoid(z) = 0.5 tanh(0.5 z)+0.5)
    # h input is 2h -> extra 0.5 on W_hh blocks.
    src = [1, 2, 0, 3]                    # f, g, i, o row-blocks in torch order
    sig = [0.5, 1.0, 0.5, 0.5]
    whha = np.zeros((P + 2, 4 * P), dtype=np.float16)
    for k in range(4):
        blk = slice(src[k] * P, (src[k] + 1) * P)
        whha[0:P, k * P:(k + 1) * P] = sig[k] * 0.5 * W_hh[blk].T
        whha[P, k * P:(k + 1) * P] = sig[k] * wih[blk]
        whha[P + 1, k * P:(k + 1) * P] = sig[k] * bsum[blk]
    return dict(wdh=wdh, wdc=wdc, udT=udT, v2=v2, whha=whha)


def kernel(H, dec_data, d_1, s_1, W_d, U_d, v_d, w_tilde_W, w_tilde_b,
           W_ih, W_hh, b_ih, b_hh, T=None):
    from concourse.bass_utils import run_bass_kernel_spmd

    H = np.asarray(H, dtype=np.float32)
    dec_data = np.asarray(dec_data, dtype=np.float32)
    d_1 = np.asarray(d_1, dtype=np.float32)
    s_1 = np.asarray(s_1, dtype=np.float32)
    W_d = np.asarray(W_d, np.float32)
    w_tilde_W = np.asarray(w_tilde_W, np.float32)
    w_tilde_b = np.asarray(w_tilde_b, np.float32)

    if "nc" not in _STATE:
        _STATE["nc"] = _build_nc()
    nc = _STATE["nc"]

    wpack = _pack_weights(
        W_d, np.asarray(U_d, np.float32),
        np.asarray(v_d, np.float32), w_tilde_W, w_tilde_b,
        np.asarray(W_ih, np.float32), np.asarray(W_hh, np.float32),
        np.asarray(b_ih, np.float32), np.asarray(b_hh, np.float32),
    )

    w1 = w_tilde_W[0, 1:M + 1]            # [64]
    w0 = w_tilde_W[0, 0]
    wb = w_tilde_b[0]

    in_maps = []
    for core in range(NCORES):
        sl = slice(core * BL, (core + 1) * BL)
        h_loc = np.ascontiguousarray(H[sl])
        ht_loc = np.ascontiguousarray(h_loc.transpose(0, 2, 1))
        st = np.concatenate(
            [2.0 * d_1[0, sl].T, 2.0 * s_1[0, sl].T], axis=0
        ).astype(np.float16)                       # [128, 4]
        # hwf[tau, b*TS+ts] = (H_b @ w~1 + w~b) [tau] + w~0*dec[b, ts]
        hwb = h_loc @ w1 + wb                      # [BL, T]
        hwf = (hwb[:, None, :] + (w0 * dec_data[sl, :TS, 0])[:, :, None])
        hwf = np.ascontiguousarray(
            hwf.transpose(2, 0, 1).reshape(T, BL * TS), dtype=np.float16)
        m = dict(wpack)
        m.update(h_l=h_loc, ht_l=ht_loc, st0=np.ascontiguousarray(st),
                 hwf=hwf)
        in_maps.append(m)

    res = run_bass_kernel_spmd(nc, in_maps, list(range(NCORES)))
    _STATE["last_results"] = res

    out = np.zeros((B, 1, P + M), dtype=np.float32)
    for core in range(NCORES):
        r = res.results[core]
        hv = r["oh"].T * 0.5                      # [4, 64]  (state was 2h)
        octx = r["octx"]
        ctx = (octx[0:M] / octx[M:M + 1]).T       # [4, 64]
        out[core * BL:(core + 1) * BL, 0, 0:P] = hv
        out[core * BL:(core + 1) * BL, 0, P:P + M] = ctx
    return out
